# revision 30
# baseline (speedup 1.0000x reference)
"""CIN (Compressed Interaction Network) kernel for Trainium2, 8-core data parallel.

Math (per batch row b, embedding dim d — R = B*D independent rows):
  layer k: cur_k[m, (b,d)] = sum_{f,g} W_k[f*G+g, m] * x0[f,(b,d)] * x_{k}[g,(b,d)]
  output  = concat_k( sum_d cur_k )    -> [B, 384]

Device strategy (per core, batch-sharded B/8 = 256 -> R = 4096 rows):
  * Everything lives feature-on-partitions: cur_k^T [128, R] etc.
  * z_k^T [(f,g), R] is built k-tile by k-tile on DVE tensor_tensor (bf16 2x):
      z-tile_f = cur_{k-1}^T * bcast(x0^T[f, :])
    The broadcast tiles come from one DRAM->SBUF DMA per chunk with a
    0-stride partition AP (DVE cannot partition-broadcast; DMA can).
  * The (f,g) contraction is standard PSUM-accumulated matmuls with weight
    k-tiles stationary, so the f-sum is free.
  * Layer 0 uses the x (x) x symmetry: W0 is host-symmetrized to the upper
    triangle (k: 1521 -> 780, padded 896); the g-side z factor is gathered
    ON-DEVICE by PE one-hot selection matmuls from the tiny per-chunk x tile
    (39x512) and evicted PSUM->SBUF by the scalar engine; the f-side factor
    (runs of repeated rows) comes via DMA.
  * Software pipeline: xs/zin1 prefetch TWO iterations ahead and the next
    chunk's L0 gather+z0 phase is hoisted to the iteration top, so the L0
    cross-engine chain (PE gathers -> ACT evictions -> DVE z0) overlaps the
    current chunk's L1 z-bursts instead of queueing behind them in the
    in-order DVE queue. bc rides the SP HWDGE ring at the iteration top
    (first 24 rows) + ACT ring at the bottom (last 15), so a ring-capacity
    block never sits ahead of compute-critical copies on either sequencer.
  * The L2 Gram operand xbd (block-diagonal, 8/9 zeros) is NOT DMA'd: a
    dense 40KB/chunk xf tile is expanded on the DVE against a constant
    placement mask, saving 2.6MB/core of DMA.
  * stage B of the L2 Gram trick runs in three pieces (chunks 0-3 at c==3,
    4-6 at c==6, chunk 7 after the loop) with the first output flush at
    c==3, so only a 32-column piece remains in the serial tail.
  * All DRAM operands are laid out chunk-major on the host so every DMA has
    large contiguous per-partition runs (big packets -> full DMA bandwidth).
"""

import sys
import types

sys.path.insert(0, "/opt/trn_rl_repo")

# The image's antenv package lacks axon_hooks; bass_utils imports it if
# BASS_TRACE is set in the environment. Seed a benign stub so that path
# degrades to "no tracing" instead of ModuleNotFoundError.
if "antenv.axon_hooks" not in sys.modules:
    _ah = types.ModuleType("antenv.axon_hooks")
    _ah.get_axon_ntff_profile_hook = lambda: None
    _ah.set_axon_ntff_profile_hook = lambda h: None
    sys.modules["antenv.axon_hooks"] = _ah

import numpy as np
import ml_dtypes

import concourse.bass as bass
import concourse.mybir as mybir
from concourse import bacc
from concourse.tile import TileContext
from concourse.bass_utils import run_bass_kernel_spmd

BF16 = ml_dtypes.bfloat16

B, F0, D = 2048, 39, 16
M = 128                      # layer width (all three layers)
NCORES = 8
BPC = B // NCORES            # batch per core = 256
R = BPC * D                  # rows per core = 4096
K0 = (F0 * (F0 + 1)) // 2    # 780 (triangular)
K0P = 896                    # padded to 7 k-tiles
NKT0 = K0P // 128            # 7
NKT = (F0 * M) // 128        # 39 k-tiles for layers 1/2

L = 512                      # bd-chunk (32 b x 16 d)
NCHUNK = R // L              # 8
BPCH = L // D                # 32 batches per chunk
NTILE = L // 128             # 4 bd-tiles of 128 rows per chunk

DT = mybir.dt.bfloat16
DTF = mybir.dt.float32

_CACHE = {}


def _build_program():
    nc = bacc.Bacc("TRN2", target_bir_lowering=False, debug=False,
                   num_devices=NCORES)

    # chunk-major layouts so per-partition DMA runs are contiguous
    xT = nc.declare_dram_parameter("xT", [NCHUNK, F0 * L], DT, isOutput=False)
    sel0 = nc.declare_dram_parameter("sel0", [F0, NKT0 * 128], DT,
                                     isOutput=False)
    selbc = nc.declare_dram_parameter("selbc", [F0, 7 * 128], DT,
                                      isOutput=False)
    zin1 = nc.declare_dram_parameter("zin1", [NCHUNK, 128, NKT0, L], DT,
                                     isOutput=False)
    w0 = nc.declare_dram_parameter("w0", [K0P, M], DT, isOutput=False)
    w1 = nc.declare_dram_parameter("w1", [F0 * M, M], DT, isOutput=False)
    w2 = nc.declare_dram_parameter("w2", [F0 * M, M], DT, isOutput=False)
    ident = nc.declare_dram_parameter("ident", [128, 128], DTF, isOutput=False)
    identb = nc.declare_dram_parameter("identb", [128, 128], DT, isOutput=False)
    xbdm = nc.declare_dram_parameter("xbdm", [128, 9 * F0], DT, isOutput=False)
    xf = nc.declare_dram_parameter("xf", [NCHUNK, 128, NTILE * F0], DT,
                                   isOutput=False)
    out = nc.declare_dram_parameter("out", [BPC, 3 * M], DTF, isOutput=True)

    with TileContext(nc) as tc:
        with (
            tc.tile_pool(name="wpool", bufs=1) as wpool,
            tc.tile_pool(name="bcast", bufs=2) as bcpool,
            tc.tile_pool(name="zin", bufs=2) as zinpool,
            tc.tile_pool(name="zt", bufs=3) as zpool,
            tc.tile_pool(name="zaux", bufs=2) as zauxpool,
            tc.tile_pool(name="cur", bufs=3) as curpool,
            tc.tile_pool(name="outp", bufs=1) as outpool,
            tc.tile_pool(name="psum", bufs=2, space="PSUM") as pspool,
            tc.tile_pool(name="pst", bufs=1, space="PSUM") as pstpool,
            tc.tile_pool(name="pstb", bufs=1, space="PSUM") as pstbpool,
            tc.tile_pool(name="psa", bufs=1, space="PSUM") as psapool,
            tc.tile_pool(name="pso2", bufs=1, space="PSUM") as pso2pool,
            tc.tile_pool(name="psg", bufs=2, space="PSUM") as psgpool,
        ):
            BCTOP = 32           # bc rows loaded at iteration top (SP ring)

            def issue_xz(c):
                """xs/zin1 prefetch, issued TWO iterations ahead so the next
                chunk's L0 gather chain never waits on DMA."""
                xs = zinpool.tile([F0, L], DT, tag="xs", name=f"xs_{c}")
                nc.sync.dma_start(
                    out=xs[:],
                    in_=xT[c : c + 1, :].rearrange("c (f l) -> (c f) l", f=F0),
                )
                zin1t = zinpool.tile([128, NKT0, L], DT, tag="zin1",
                                     name=f"zin1_{c}")
                nc.sync.dma_start(out=zin1t[:, :4, :], in_=zin1[c, :, :4, :])
                nc.sync.dma_start(out=zin1t[:, 4:, :], in_=zin1[c, :, 4:, :])
                return xs, zin1t

            NBC = 32             # bc rows via DMA; the last 7 via PE gathers

            def issue_chunk_top(c):
                """bc/xf prefetch on the SP ring, one iteration ahead."""
                xft = zinpool.tile([128, NTILE * F0], DT, tag="xf",
                                   name=f"xf_{c}")
                nc.sync.dma_start(out=xft[:], in_=xf[c])
                bc = bcpool.tile([128, NBC, L], DT, tag="bc", name=f"bc_{c}")
                xTv = xT[c : c + 1, :].to_broadcast((128, F0 * L)).rearrange(
                    "p (f l) -> p f l", f=F0
                )
                for f in range(0, BCTOP, 8):
                    nc.sync.dma_start(out=bc[:, f : f + 8, :],
                                      in_=xTv[:, f : f + 8, :])
                return bc, xft

            def issue_chunk_bottom(c, bc):
                """Iteration-bottom bc tail on the ACT ring — emitted after
                the iteration's PSUM-eviction copies so a ring-capacity block
                here cannot delay them."""
                xTv = xT[c : c + 1, :].to_broadcast((128, F0 * L)).rearrange(
                    "p (f l) -> p f l", f=F0
                )
                nc.scalar.dma_start(out=bc[:, BCTOP:NBC, :],
                                    in_=xTv[:, BCTOP:NBC, :])

            def pbc_gather(c, xs):
                """PE-built broadcast rows f = 32..38 for chunk c: one-hot
                selection matmuls replicate xs rows across all partitions."""
                pbc = bcpool.tile([128, 7, L], DT, tag="pbc", name=f"pbc_{c}")
                for j in range(7):
                    pgb = psgpool.tile([128, L], DTF, tag="psg",
                                       name=f"pgb_{c}_{j}")
                    nc.tensor.matmul(pgb[:], selbcs[:, j, :], xs[:],
                                     start=True, stop=True)
                    nc.scalar.copy(pbc[:, j, :], pgb[:])
                return pbc

            G0 = [4, 3]          # layer-0 k-tile TT groups (sum NKT0)
            G12 = [4, 8, 8, 8, 8, 3]  # layer-1/2 f-groups (sum F0)

            # ---- startup: everything on the SP ring in priority order.
            # The ACT ring starts empty so the scalar engine's first PSUM
            # evictions are never stuck behind a ring-capacity block.
            w0s = wpool.tile([128, NKT0, M], DT, tag="w0")
            nc.sync.dma_start(out=w0s[:], in_=w0.rearrange("(t p) m -> p t m", p=128))
            sel0s = wpool.tile([F0, NKT0, 128], DT, tag="sel0")
            nc.sync.dma_start(out=sel0s[:],
                              in_=sel0.rearrange("f (t p) -> f t p", t=NKT0))
            selbcs = wpool.tile([F0, 7, 128], DT, tag="selbc")
            nc.sync.dma_start(out=selbcs[:],
                              in_=selbc.rearrange("f (t p) -> f t p", t=7))
            xbdms = wpool.tile([128, 9 * F0], DT, tag="xbdm")
            nc.sync.dma_start(out=xbdms[:], in_=xbdm[:])
            # xs/zin1 for chunk 0, then w1 (needed with the first bc group),
            # then bc 0 top groups; xbd_0/identities trail them.
            w1s = wpool.tile([128, NKT, M], DT, tag="w1")
            w2s = wpool.tile([128, NKT, M], DT, tag="w2")
            idb = wpool.tile([128, 128], DT, tag="identb")
            ids = wpool.tile([128, 128], DTF, tag="ident")
            xs0 = zinpool.tile([F0, L], DT, tag="xs", name="xs_0")
            nc.sync.dma_start(
                out=xs0[:],
                in_=xT[0:1, :].rearrange("c (f l) -> (c f) l", f=F0),
            )
            zin10 = zinpool.tile([128, NKT0, L], DT, tag="zin1", name="zin1_0")
            nc.sync.dma_start(out=zin10[:, :4, :], in_=zin1[0, :, :4, :])
            nc.sync.dma_start(out=zin10[:, 4:, :], in_=zin1[0, :, 4:, :])
            # chunk-1 xs/zin1 ride right behind chunk 0's critical set so the
            # chunk-1 gather chain starts during iteration 0, not after the
            # whole startup pile has drained.
            xz1 = issue_xz(1)
            xf0 = zinpool.tile([128, NTILE * F0], DT, tag="xf", name="xf_0")
            nc.sync.dma_start(out=xf0[:], in_=xf[0])
            nc.sync.dma_start(out=w1s[:], in_=w1.rearrange("(t p) m -> p t m", p=128))
            bc0 = bcpool.tile([128, NBC, L], DT, tag="bc", name="bc_0")
            xTv0 = xT[0:1, :].to_broadcast((128, F0 * L)).rearrange(
                "p (f l) -> p f l", f=F0
            )
            for f in range(0, BCTOP, 8):
                nc.sync.dma_start(out=bc0[:, f : f + 8, :],
                                  in_=xTv0[:, f : f + 8, :])
            nc.sync.dma_start(out=idb[:], in_=identb[:])
            nc.sync.dma_start(out=ids[:], in_=ident[:])
            pref = (bc0, xs0, zin10, xf0)
            # chunk-1 xs/zin1 right behind chunk 0's critical set

            # per-layer output accumulators [128 m, BPC] fp32
            outacc = [
                outpool.tile([128, BPC], DTF, tag=f"oacc{k}", name=f"oacc{k}")
                for k in range(3)
            ]

            def do_l0_gather(c, xs, zin1t):
                """L0 phase A for chunk c: PE one-hot gathers + ACT evictions
                + the DVE z0 products. Emitted at the TOP of iteration c-1 so
                it overlaps iteration c-1's L1 z-bursts."""
                zin0t = zinpool.tile([128, NKT0, L], DT, tag="zin0",
                                     name=f"zin0_{c}")
                for t in range(NKT0):
                    pg0 = psgpool.tile([128, L], DTF, tag="psg",
                                       name=f"pg0_{c}_{t}")
                    nc.tensor.matmul(pg0[:], sel0s[:, t, :], xs[:],
                                     start=True, stop=True)
                    nc.scalar.copy(zin0t[:, t, :], pg0[:])
                z0ts = []
                kt = 0
                for gsz in G0:
                    z0t = zauxpool.tile([128, gsz, L], DT, tag="z0",
                                     name=f"z0_{c}_{kt}")
                    nc.vector.tensor_mul(
                        z0t[:],
                        zin0t[:, kt : kt + gsz, :],
                        zin1t[:, kt : kt + gsz, :],
                    )
                    z0ts.append(z0t)
                    kt += gsz
                return z0ts

            def do_l0(c, z0ts):
                """L0 phase B for chunk c: the W0 contraction + cur0/out0."""
                ps0 = pspool.tile([128, L], DTF, tag="ps", name=f"ps0_{c}")
                kt = 0
                for gi_, gsz in enumerate(G0):
                    z0t = z0ts[gi_]
                    for j in range(gsz):
                        nc.tensor.matmul(
                            ps0[:],
                            w0s[:, kt + j, :],
                            z0t[:, j, :],
                            start=(kt + j == 0),
                            stop=(kt + j == NKT0 - 1),
                        )
                    kt += gsz
                cur0 = curpool.tile([128, L], DT, tag="cur0", name=f"cur0_{c}")
                nc.scalar.copy(cur0[:], ps0[:])
                nc.vector.tensor_reduce(
                    outacc[0][:, c * BPCH : (c + 1) * BPCH],
                    cur0[:].rearrange("p (b d) -> p b d", d=D),
                    axis=mybir.AxisListType.X,
                    op=mybir.AluOpType.add,
                )
                return cur0

            def do_layer(c, lyr, ws, prev, bc, pbc):
                """Layer 1/2 for chunk c: z = prev (x) bc, matmul, reduce."""
                ps = pspool.tile([128, L], DTF, tag="ps", name=f"ps{lyr}_{c}")
                f = 0
                for gsz in G12:
                    zt = zpool.tile([128, gsz, L], DT, tag="z",
                                    name=f"z{lyr}_{c}_{f}")
                    src2 = (bc[:, f : f + gsz, :] if f + gsz <= NBC
                            else pbc[:, :, :])
                    nc.vector.tensor_mul(
                        zt[:],
                        prev[:].unsqueeze(1).to_broadcast((128, gsz, L)),
                        src2,
                    )
                    for j in range(gsz):
                        nc.tensor.matmul(
                            ps[:],
                            ws[:, f + j, :],
                            zt[:, j, :],
                            start=(f + j == 0),
                            stop=(f + j == F0 - 1),
                        )
                    f += gsz
                cur1 = curpool.tile([128, L], DT, tag="cur1",
                                    name=f"cur1_{c}")
                nc.scalar.copy(cur1[:], ps[:])
                nc.vector.tensor_reduce(
                    outacc[lyr][:, c * BPCH : (c + 1) * BPCH],
                    cur1[:].rearrange("p (b d) -> p b d", d=D),
                    axis=mybir.AxisListType.X,
                    op=mybir.AluOpType.add,
                )
                return cur1

            # layer 2 via the d-contraction (Gram) trick: the final output
            # only needs sum_d cur2, and
            #   out2[b, m] = sum_{f,g} W2[fg, m] * P12[b, f, g],
            #   P12[b, f, g] = sum_d x[b, f, d] * cur1[b, g, d].
            # P12 is built on the PE with a block-diagonal x operand (bd rows
            # on partitions, contraction over the 16 d's of each batch).
            p12 = outpool.tile([128, NCHUNK * NTILE * 9 * F0], DT, tag="p12")

            def do_l2p(c, cur1, xft):
                xbdt = zauxpool.tile([128, NTILE, 9 * F0], DT, tag="xbdt",
                                  name=f"xbdt_{c}")
                nc.gpsimd.tensor_mul(
                    xbdt[:].rearrange("p t (s f) -> p t s f", f=F0),
                    xft[:].rearrange("p (t f) -> p t f", f=F0)
                         .unsqueeze(2).to_broadcast((128, NTILE, 9, F0)),
                    xbdms[:].rearrange("p (s f) -> p s f", f=F0)
                          .unsqueeze(1).to_broadcast((128, NTILE, 9, F0)),
                )
                for t in range(NTILE):
                    pstc = pstbpool.tile([128, 128], DT, tag="pstb",
                                        name=f"pstc_{c}_{t}")
                    nc.tensor.transpose(
                        pstc[:], cur1[:, t * 128 : (t + 1) * 128], idb[:]
                    )
                    c1bd = curpool.tile([128, 128], DT, tag="c1bd",
                                        name=f"c1bd_{c}_{t}")
                    nc.scalar.copy(c1bd[:], pstc[:])
                    psa = psapool.tile([128, 9 * F0], DTF, tag="psa",
                                       name=f"psa_{c}_{t}")
                    nc.tensor.matmul(
                        psa[:], c1bd[:], xbdt[:, t, :],
                        start=True, stop=True,
                    )
                    off = (c * NTILE + t) * 9 * F0
                    nc.scalar.copy(p12[:, off : off + 9 * F0], psa[:])

            def flush_out(h):
                for k in range(3):
                    pst = pstpool.tile([128, 128], DTF, tag="pst",
                                       name=f"pst_{k}_{h}")
                    nc.tensor.transpose(
                        pst[:], outacc[k][:, h * 128 : (h + 1) * 128], ids[:]
                    )
                    ot = curpool.tile([128, 128], DTF, tag="otile",
                                      name=f"ot_{k}_{h}")
                    nc.scalar.copy(ot[:], pst[:])
                    nc.sync.dma_start(
                        out=out[h * 128 : (h + 1) * 128, k * M : (k + 1) * M],
                        in_=ot[:],
                    )

            # stage B of the L2 Gram trick, over a half-range of chunks:
            # out2^T[m, b-half] = sum_f w2_f^T @ p12[:, (c, t, b, f)]
            pso2 = pso2pool.tile([128, BPC], DTF, tag="pso2")
            p12v = p12[:].rearrange("p (ct s f) -> p ct s f", s=9, f=F0)

            def stage_b(c0, c1):
                nt0, nt1 = c0 * NTILE, c1 * NTILE
                b0, b1 = c0 * BPCH, c1 * BPCH
                for f in range(F0):
                    nc.tensor.matmul(
                        pso2[:, b0:b1], w2s[:, f, :],
                        p12v[:, nt0:nt1, 0:8, f],
                        start=(f == 0), stop=(f == F0 - 1),
                    )
                nc.scalar.copy(outacc[2][:, b0:b1], pso2[:, b0:b1])

            # software pipeline: chunk c+1's L0 runs between chunk c's L1 and
            # the L2 P-build so the PE/DVE never drain at chunk boundaries.
            bc_c, xs_c, zin1_c, xft_c = pref
            z0_c = do_l0_gather(0, xs_c, zin1_c)
            pbc_c = pbc_gather(0, xs_c)
            cur0_c = do_l0(0, z0_c)
            issue_chunk_bottom(0, bc_c)
            xs_n, zin1_n = xz1
            z0_n = do_l0_gather(1, xs_n, zin1_n)
            pbc_n = pbc_gather(1, xs_n)
            for c in range(NCHUNK):
                if c + 2 < NCHUNK:
                    xs_2, zin1_2 = issue_xz(c + 2)
                if c + 1 < NCHUNK:
                    bc_n, xft_n = issue_chunk_top(c + 1)
                cur1_c = do_layer(c, 1, w1s, cur0_c, bc_c, pbc_c)
                if c + 1 < NCHUNK:
                    cur0_c = do_l0(c + 1, z0_n)
                if c + 2 < NCHUNK:
                    z0_n = do_l0_gather(c + 2, xs_2, zin1_2)
                    pbc_2 = pbc_gather(c + 2, xs_2)
                do_l2p(c, cur1_c, xft_c)
                if c + 1 < NCHUNK:
                    issue_chunk_bottom(c + 1, bc_n)
                if c == 0:
                    nc.scalar.dma_start(out=w2s[:],
                                        in_=w2.rearrange("(t p) m -> p t m", p=128))
                if c == 3:
                    stage_b(0, 4)
                    flush_out(0)
                if c + 1 < NCHUNK:
                    bc_c, xft_c, pbc_c = bc_n, xft_n, pbc_n
                if c + 2 < NCHUNK:
                    pbc_n = pbc_2

            # stage B second half; the first half was emitted inside the loop.
            stage_b(4, 8)
            flush_out(1)


    nc.compile()
    return nc


def _host_prep(inputs, f0, f1, f2):
    """Per-core input maps. Pure layout/cast/index-gather, no FLOP offload
    (except the W0 symmetrization, which is weight preprocessing)."""
    x = np.asarray(inputs)

    # symmetrized triangular W0: rows (f, g) f<=g
    f0n = np.asarray(f0).reshape(F0, F0, M)
    fi, gi = np.triu_indices(F0)
    w0t = f0n[fi, gi] + np.where((fi != gi)[:, None], f0n[gi, fi], 0.0)
    w0 = np.zeros((K0P, M), dtype=BF16)
    w0[:K0] = w0t.astype(BF16)

    w1 = np.asarray(f1).astype(BF16)
    w2 = np.asarray(f2).astype(BF16)
    ident = np.eye(128, dtype=np.float32)
    identb = np.eye(128, dtype=BF16)

    # layer-0 z-factor gathers (triangular, k-row = tile*128 + p): both sides
    # are device-side one-hot selection matmuls sel0[f, k] = (gidx[k] == f),
    # sel1[f, k] = (fidx[k] == f).
    sel0np = np.zeros((F0, K0P), dtype=BF16)
    sel0np[gi, np.arange(K0)] = 1
    selbcnp = np.zeros((F0, 7 * 128), dtype=BF16)
    for j in range(7):
        selbcnp[32 + j, j * 128 : (j + 1) * 128] = 1
    fidx = np.zeros(K0P, np.int64)
    fidx[:K0] = fi
    valid = (np.arange(K0P) < K0).astype(BF16)[:, None]

    # block-diagonal placement mask for the L2 Gram trick: row p keeps the
    # s = p//16 block of the 9*F0 columns.
    p = np.arange(128)
    xbdmnp = np.zeros((128, 9 * F0), dtype=BF16)
    for s in range(8):
        xbdmnp[p // 16 == s, s * F0 : (s + 1) * F0] = 1

    maps = []
    for c in range(NCORES):
        xs = x[c * BPC : (c + 1) * BPC]                    # [256, 39, 16]
        xTf = np.ascontiguousarray(
            xs.transpose(1, 0, 2).reshape(F0, R)
        ).astype(BF16)                                     # [39, R]
        # chunk-major broadcast source: [NCHUNK, F0*L]
        xTc = np.ascontiguousarray(
            xTf.reshape(F0, NCHUNK, L).transpose(1, 0, 2)
        ).reshape(NCHUNK, F0 * L)
        # layer-0 f-side factor [K0P, R] -> chunk-major [NCHUNK, 128, NKT0, L]
        z0b = (xTf[fidx] * valid).reshape(NKT0, 128, NCHUNK, L)
        zin1c = np.ascontiguousarray(z0b.transpose(2, 1, 0, 3))
        # dense bd-major x for the L2 Gram trick: [NCHUNK, 128, NTILE*F0]
        xbd_full = xs.transpose(0, 2, 1).reshape(R, F0).astype(BF16)
        xfc = np.ascontiguousarray(
            xbd_full.reshape(NCHUNK, NTILE, 128, F0).transpose(0, 2, 1, 3)
        ).reshape(NCHUNK, 128, NTILE * F0)
        maps.append(
            dict(xT=xTc, sel0=sel0np, selbc=selbcnp, zin1=zin1c, w0=w0,
                 w1=w1, w2=w2,
                 ident=ident, identb=identb, xbdm=xbdmnp, xf=xfc)
        )
    return maps


def kernel(**inputs) -> np.ndarray:
    if "nc" not in _CACHE:
        _CACHE["nc"] = _build_program()
    nc = _CACHE["nc"]
    maps = _host_prep(inputs["inputs"], inputs["f0"], inputs["f1"], inputs["f2"])
    res = run_bass_kernel_spmd(nc, maps, list(range(NCORES)))
    return np.concatenate([res.results[c]["out"] for c in range(NCORES)], axis=0)


if __name__ == "__main__":
    rng = np.random.default_rng(0)
    ins = {
        "inputs": rng.standard_normal((B, F0, D), dtype=np.float32),
        "f0": (rng.standard_normal((F0 * F0, M)) * 0.05).astype(np.float32),
        "f1": (rng.standard_normal((F0 * M, M)) * 0.05).astype(np.float32),
        "f2": (rng.standard_normal((F0 * M, M)) * 0.05).astype(np.float32),
    }
    out = kernel(**ins)
    print("out", out.shape, out.dtype)

